# revision 41
# baseline (speedup 1.0000x reference)
"""PacConv2d (BlockPAC) Trainium2 kernel.

nn_BlockPAC: guide-adaptive 3x3 convolution (PAC) + bias + relu.
  kernel[b,p,h,w] = exp(-0.5 * sum_cg (guide_tap_p - guide_center)^2)
  out[b,o,h,w]    = relu(bias[o] + sum_{c,p} x_tap_p[b,c,h,w] * kernel[b,p,h,w]
                                            * weight[o,c,p])

Sharding: data-parallel over batch B=8 across the 8 NeuronCores (one sample
per core). No collectives.

Host side does layout only (zero-pad + im2col tap stacking + bf16 cast); all
arithmetic (center replication, diff, square, channel reduction, exp, the
adaptive multiply, the weight contraction, bias, relu) runs on device.

Design notes (timeline-model driven; modeled ~83us/core, DMA-bound at ~87%):
  * all-bf16 data path. fp32 matmuls cost 4 cycles/row on TRN2 vs 1 for
    bf16, so the center tap also runs bf16 (rel-err ~3e-3, gate is 2e-2).
    bf16 output too; host upcasts.
  * im2col x-stack shipped as xs4 (128, 4, H, W): partition = 8 taps x 16
    chans, free dim packs the 4 channel groups; one DMA per group per
    16-row block keeps descriptors big (4KB runs) and lets each group's
    adaptive multiply start as soon as its slice lands.
  * guide path ships only the 8-tap stack gs (128,H,W) and the raw center
    gpc (16,H,W): the 16x center replication (128,H,W) is rebuilt on
    device by a K=16 PE matmul with a 0/1 replication lhsT, evicted
    PSUM->SBUF by the Activation engine (GPSIMD cannot touch PSUM).
    That trades 3.5MB of DMA for ~7us of idle PE+ACT time.
  * diff and square both run on DVE back-to-back (bf16 2x mode, no
    cross-engine hop); sub is in-place into the gs tile.
  * D-matmul: lhsT(128,128) block-diag(-0.5) x sq -> PSUM computes
    -0.5*sum_cg per tap AND replicates it across the tap's 16 partitions.
  * E = exp(D) (ACT, PSUM->SBUF, bf16); y[g] = xs4[g]*E (DVE 2x).
  * out PSUM += sum_g W_g^T y_g (4 bf16 matmuls, K=128) + Wc^T x_center
    (1 bf16 matmul, K=64), lhsT-major so Ldweights reloads stay rare.
    The two 8-row half-blocks alternate partition halves of ONE PSUM tile
    (banks are the scarce resource: 2 gcr + 4 dps + 2 out = 8).
  * relu(+bias) per half-block -> bf16, out-DMA per half-block issued on
    the ACT queue right after its relu (drains early, keeps the DMA
    engines gapless; queue placement matters: each DMA costs ~700ns of
    DGE delay on its issuing sequencer).
  * emission order is tuned for the 4-deep per-engine wait queues: the
    next block's center-replication chunks are emitted mid-block as PE
    filler, at CH=4-row granularity so their PSUM waits never head-block
    ready D-matmuls.
"""

import contextlib
import sys

import numpy as np

sys.path.insert(0, "/opt/trn_rl_repo")

import ml_dtypes

from concourse import bass, mybir, tile
from concourse.bass_utils import run_bass_kernel_spmd

# ---------------------------------------------------------------- constants
B, CIN, COUT, CG, H, W = 8, 64, 64, 16, 128, 128
KS, PAD = 3, 1
HP, WP = H + 2 * PAD, W + 2 * PAD  # 130, 130
NCORES = 8

R = 16                      # output rows per block
HGRP = 8                    # rows per psum group
CH = 4                      # output rows per matmul chunk (N = 4*128 = 512)

# non-center taps p=3i+j, p != 4, in reference order
TAPS = [(p // 3, p % 3) for p in range(9) if p != 4]
NT = len(TAPS)              # 8
CTR_I, CTR_J = 1, 1

# constants tile layout (free-dim columns)
CST_WSTK = 0                # [0:256)   wstk: col 64*g + o, part 16*t + c
CST_WCTR = 256              # [256:320) wctr: col o, part c  (64 parts)
CST_LHSD = 320              # [320:448) lhsd: block-diag -0.5
CST_BIAS = 448              # [448:449) bias  (64 parts)
CST_REP = 449               # [449:577) rep16: center-replication lhsT (16 parts)
CST_W = 577

F32 = mybir.dt.float32
BF = mybir.dt.bfloat16
NPBF = ml_dtypes.bfloat16

_cache = {}


# ---------------------------------------------------------------- bass build
def _build_nc():
    nc = bass.Bass(
        "TRN2",
        target_bir_lowering=False,
        debug=False,
        enable_asserts=False,
        num_devices=NCORES,
    )

    xs4_d = nc.dram_tensor("xs4", [128, 4, H, W], BF, kind="ExternalInput").ap()
    gs_d = nc.dram_tensor("gs", [128, H, W], BF, kind="ExternalInput").ap()
    gpc_d = nc.dram_tensor("gpc", [CG, H, W], BF, kind="ExternalInput").ap()
    xpb_d = nc.dram_tensor("xpb", [CIN, HP, WP], BF, kind="ExternalInput").ap()
    cst_d = nc.dram_tensor("cst", [128, CST_W], BF, kind="ExternalInput").ap()
    out_d = nc.dram_tensor("out", [COUT, H, W], BF, kind="ExternalOutput").ap()

    NBLK = H // R  # 8 blocks of 16 rows

    with tile.TileContext(nc) as tc:
        with contextlib.ExitStack() as ctx:
            cstp = ctx.enter_context(tc.tile_pool(name="cstp", bufs=1))
            bin_ = ctx.enter_context(tc.tile_pool(name="bin", bufs=3))
            blk = ctx.enter_context(tc.tile_pool(name="blk", bufs=3))
            cnk = ctx.enter_context(tc.tile_pool(name="cnk", bufs=3))
            psg = ctx.enter_context(tc.tile_pool(name="psg", bufs=2, space="PSUM"))
            psd = ctx.enter_context(tc.tile_pool(name="psd", bufs=4, space="PSUM"))
            pso = ctx.enter_context(tc.tile_pool(name="pso", bufs=1, space="PSUM"))

            cst = cstp.tile([128, CST_W], BF, name="cst")
            nc.sync.dma_start(cst[:], cst_d[:])
            wstk = [cst[:, CST_WSTK + 64 * g : CST_WSTK + 64 * (g + 1)]
                    for g in range(4)]
            wctr = cst[0:CIN, CST_WCTR : CST_WCTR + COUT]
            lhsd = cst[:, CST_LHSD : CST_LHSD + 128]
            rep16 = cst[0:CG, CST_REP : CST_REP + 128]
            bias32 = cstp.tile([COUT, 1], F32, name="bias32")
            nc.vector.tensor_copy(bias32[:], cst[0:COUT, CST_BIAS : CST_BIAS + 1])

            # whole-image replicated guide-center buffer. Two half-image gpc
            # DMAs (issued up front / at the halfway block); the PE-matmul ->
            # Pool-evict replication chunks are interleaved into each block's
            # phase A so they never head-block the in-order PE queue.
            gpcw = ctx.enter_context(tc.tile_pool(name="gpcw", bufs=1))
            gcs = gpcw.tile([128, H, W], BF, name="gcs")
            H2 = H // 2
            gpc2 = [gpcw.tile([CG, H2, W], BF, name=f"gpc2{i}") for i in range(2)]
            for i in range(2):
                nc.sync.dma_start(gpc2[i][:], gpc_d[:, H2 * i : H2 * (i + 1), :])

            def gcs_chunks(b):
                r0 = R * b
                gp = gpc2[r0 // H2]
                for v in range(R // CH):
                    gcr = psg.tile([128, CH, W], F32, name="gcr")
                    nc.tensor.matmul(
                        gcr[:],
                        rep16,
                        gp[:, (r0 % H2) + CH * v : (r0 % H2) + CH * (v + 1), :],
                        start=True,
                        stop=True,
                    )
                    nc.scalar.activation(
                        gcs[:, r0 + CH * v : r0 + CH * (v + 1), :],
                        gcr[:],
                        mybir.ActivationFunctionType.Copy,
                    )

            def block(b):
                r0 = R * b
                gs = bin_.tile([128, R, W], BF, name="gs")
                nc.gpsimd.dma_start(gs[:], gs_d[:, r0 : r0 + R, :])
                xs4 = bin_.tile([128, 4, R, W], BF, name="xs4")
                for g in range(4):
                    nc.sync.dma_start(
                        xs4[:, g, :, :], xs4_d[:, g, r0 : r0 + R, :]
                    )
                xpb = bin_.tile([CIN, R + 2, WP], BF, name="xpb")
                nc.sync.dma_start(xpb[:], xpb_d[:, r0 : r0 + R + 2, :])

                nc.vector.tensor_sub(gs[:], gs[:], gcs[:, r0 : r0 + R, :])
                sq = blk.tile([128, R, W], BF, name="sq")
                nc.vector.tensor_mul(sq[:], gs[:], gs[:])
                e8 = blk.tile([128, R, W], BF, name="e8")
                osb = blk.tile([COUT, R, W], BF, name="osb")
                opsb = pso.tile([128, HGRP, W], F32, name="opsb")

                for h in range(R // HGRP):
                    hr = HGRP * h
                    for q in range(HGRP // CH):
                        dps = psd.tile([128, CH, W], F32, name="dps")
                        nc.tensor.matmul(
                            dps[:],
                            lhsd,
                            sq[:, hr + CH * q : hr + CH * (q + 1), :],
                            start=True,
                            stop=True,
                        )
                        nc.scalar.activation(
                            e8[:, hr + CH * q : hr + CH * (q + 1), :],
                            dps[:],
                            mybir.ActivationFunctionType.Exp,
                        )

                    ys = []
                    for g in range(4):
                        yt = cnk.tile([128, HGRP, W], BF, name=f"y{g}")
                        nc.vector.tensor_mul(
                            yt[:],
                            xs4[:, g, hr : hr + HGRP, :],
                            e8[:, hr : hr + HGRP, :],
                        )
                        ys.append(yt)

                    # hgroups alternate partition halves of one PSUM tile
                    ops = opsb[64 * h : 64 * h + COUT, :, :]
                    # lhsT-major order: one Ldweights per stationary tensor
                    for g in range(4):
                        for q in range(HGRP // CH):
                            nc.tensor.matmul(
                                ops[:, CH * q : CH * (q + 1), :],
                                wstk[g],
                                ys[g][:, CH * q : CH * (q + 1), :],
                                start=(g == 0),
                                stop=False,
                            )
                    for q in range(HGRP // CH):
                        nc.tensor.matmul(
                            ops[:, CH * q : CH * (q + 1), :],
                            wctr,
                            xpb[:, 1 + hr + CH * q : 1 + hr + CH * (q + 1),
                                1 : 1 + W],
                            start=False,
                            stop=True,
                        )

                    nc.scalar.activation(
                        osb[:, hr : hr + HGRP, :],
                        ops[:],
                        mybir.ActivationFunctionType.Relu,
                        bias=bias32[:],
                    )
                    nc.scalar.dma_start(
                        out_d[:, r0 + hr : r0 + hr + HGRP, :],
                        osb[:, hr : hr + HGRP, :],
                    )

                    if h == 0 and b + 1 < NBLK:
                        # next block's guide-center replication as PE filler
                        gcs_chunks(b + 1)

            gcs_chunks(0)
            for b in range(NBLK):
                block(b)

    _split_waits(nc)
    return nc


_SKIP_SPLIT = {"InstCall", "InstUnconditionalBranch", "InstEventSemaphore"}


def _split_waits(nc):
    """Walrus's PSEUDO_DMA_DIRECT2D (and friends) carry a single sync-wait
    slot; Tile can attach several. Peel extra waits onto single-wait
    EventSemaphore instructions on the same engine immediately before the
    instruction (classic raw-bass wait-then-issue pattern)."""
    nopctr = [0]
    scratch_id = max(int(k) for k in nc.m.ant_sem_names) + 1
    nc.m.ant_sem_names[str(scratch_id)] = ["waitnop_scratch"]

    def mk_nop(engine, wait):
        nopctr[0] += 1
        nop = mybir.InstEventSemaphore(
            name=f"I-waitnop-{nopctr[0]}", ins=[], outs=[]
        )
        nop.engine = engine
        upd = mybir.SyncUpdate(
            sync_type="semaphore",
            id=scratch_id,
            ant_name="waitnop_scratch",
            update_mode="sem-add-imm",
            update_value=0,
            update_reg=None,
        )
        nop.sync_info = mybir.SyncInfo(on_wait=[wait], on_update=[upd])
        return nop

    for f in nc.m.functions:
        for blk_ in f.blocks:
            out = []
            for inst in blk_.instructions:
                si = inst.sync_info
                if (
                    si is not None
                    and si.on_wait
                    and len(si.on_wait) > 1
                    and type(inst).__name__ not in _SKIP_SPLIT
                ):
                    waits = list(si.on_wait)
                    for w in waits[:-1]:
                        out.append(mk_nop(inst.engine, w))
                    inst.sync_info = mybir.SyncInfo(
                        on_wait=[waits[-1]], on_update=list(si.on_update)
                    )
                out.append(inst)
            blk_.instructions[:] = out


def _get_nc():
    if "nc" not in _cache:
        _cache["nc"] = _build_nc()
    return _cache["nc"]


# ---------------------------------------------------------------- host side
def _prep_inputs(x, guide, weight, bias):
    x = np.asarray(x, dtype=np.float32)
    guide = np.asarray(guide, dtype=np.float32)
    weight = np.asarray(weight, dtype=np.float32)
    bias = np.asarray(bias, dtype=np.float32)

    xpb = np.pad(x, ((0, 0), (0, 0), (PAD, PAD), (PAD, PAD))).astype(NPBF)
    gp = np.pad(guide, ((0, 0), (0, 0), (PAD, PAD), (PAD, PAD))).astype(NPBF)

    # pre-stacked im2col tap tensors (pure layout, no arithmetic)
    xs4 = np.empty((B, 128, 4, H, W), dtype=NPBF)
    gs = np.empty((B, 128, H, W), dtype=NPBF)
    for t, (ti, tj) in enumerate(TAPS):
        for g in range(4):
            xs4[:, 16 * t : 16 * t + 16, g] = xpb[
                :, 16 * g : 16 * g + 16, ti : ti + H, tj : tj + W
            ]
        gs[:, 16 * t : 16 * t + 16] = gp[:, :, ti : ti + H, tj : tj + W]
    gpc = np.ascontiguousarray(gp[:, :, CTR_I : CTR_I + H, CTR_J : CTR_J + W])

    cst = np.zeros((128, CST_W), dtype=np.float32)
    for g in range(4):
        for t, (ti, tj) in enumerate(TAPS):
            cst[16 * t : 16 * t + 16,
                CST_WSTK + 64 * g : CST_WSTK + 64 * (g + 1)] = weight[
                :, 16 * g : 16 * g + 16, ti, tj
            ].T
    cst[0:CIN, CST_WCTR : CST_WCTR + COUT] = weight[:, :, CTR_I, CTR_J].T
    for t in range(NT):
        cst[16 * t : 16 * t + 16, CST_LHSD + 16 * t : CST_LHSD + 16 * t + 16] = (
            -0.5 * np.eye(16) @ np.ones((16, 16))
        )
    cst[0:COUT, CST_BIAS] = bias
    for c in range(CG):
        for t in range(NT):
            cst[c, CST_REP + 16 * t + c] = 1.0
    cst = cst.astype(NPBF)

    in_maps = []
    for i in range(NCORES):
        in_maps.append(
            {
                "xs4": np.ascontiguousarray(xs4[i]),
                "gs": np.ascontiguousarray(gs[i]),
                "gpc": gpc[i],
                "xpb": np.ascontiguousarray(xpb[i]),
                "cst": cst,
            }
        )
    return in_maps


def _run(in_maps, trace=False, **kw):
    nc = _get_nc()
    last = None
    for attempt in range(3):
        try:
            res = run_bass_kernel_spmd(
                nc, in_maps, list(range(NCORES)), trace=trace, **kw
            )
            break
        except Exception as e:  # wedged device: wait and retry
            last = e
            import time as _t

            _t.sleep(20 * (attempt + 1))
    else:
        raise last
    out = np.stack([res.results[i]["out"] for i in range(NCORES)], axis=0)
    return out.astype(np.float32), res


def kernel(x, guide, weight, bias):
    in_maps = _prep_inputs(x, guide, weight, bias)
    out, _ = _run(in_maps)
    return out


# revision 47
# speedup vs baseline: 1.0072x; 1.0072x over previous
"""PacConv2d (BlockPAC) Trainium2 kernel.

nn_BlockPAC: guide-adaptive 3x3 convolution (PAC) + bias + relu.
  kernel[b,p,h,w] = exp(-0.5 * sum_cg (guide_tap_p - guide_center)^2)
  out[b,o,h,w]    = relu(bias[o] + sum_{c,p} x_tap_p[b,c,h,w] * kernel[b,p,h,w]
                                            * weight[o,c,p])

Sharding: data-parallel over batch B=8 across the 8 NeuronCores (one sample
per core). No collectives.

Host side does layout only (zero-pad + im2col tap stacking + bf16 cast); all
arithmetic (center replication, diff, square, channel reduction, exp, the
adaptive multiply, the weight contraction, bias, relu) runs on device.

Design notes (timeline-model driven; modeled ~83us/core, DMA-bound at ~87%):
  * all-bf16 data path. fp32 matmuls cost 4 cycles/row on TRN2 vs 1 for
    bf16, so the center tap also runs bf16 (rel-err ~3e-3, gate is 2e-2).
    bf16 output too; host upcasts.
  * im2col x-stack shipped as xs4 (128, 4, H, W): partition = 8 taps x 16
    chans, free dim packs the 4 channel groups; one DMA per 8-row block
    (2KB runs) -- finer blocks shrink the tail drain, and one merged DMA
    per block keeps the HWDGE descriptor engine (625ns/DMA) off the
    critical path.
  * guide path ships only the 8-tap stack gs (128,H,W) and the raw center
    gpc (16,H,W): the 16x center replication (128,H,W) is rebuilt on
    device by a K=16 PE matmul with a 0/1 replication lhsT, evicted
    PSUM->SBUF by the Activation engine (GPSIMD cannot touch PSUM).
    That trades 3.5MB of DMA for ~7us of idle PE+ACT time.
  * diff and square both run on DVE back-to-back (bf16 2x mode, no
    cross-engine hop); sub is in-place into the gs tile.
  * D-matmul: lhsT(128,128) block-diag(-0.5) x sq -> PSUM computes
    -0.5*sum_cg per tap AND replicates it across the tap's 16 partitions.
  * E = exp(D) (ACT, PSUM->SBUF, bf16); y[g] = xs4[g]*E (DVE 2x).
  * out PSUM += sum_g W_g^T y_g (4 bf16 matmuls, K=128) + Wc^T x_center
    (1 bf16 matmul, K=64), lhsT-major so Ldweights reloads stay rare.
    Consecutive 8-row blocks alternate partition halves of ONE hoisted
    PSUM tile (banks are the scarce resource: 2 gcr + 4 dps + 2 out = 8).
  * relu(+bias) per block -> bf16, out-DMA per block issued on
    the ACT queue right after its relu (drains early, keeps the DMA
    engines gapless; queue placement matters: each DMA costs ~700ns of
    DGE delay on its issuing sequencer).
  * emission order is tuned for the 4-deep per-engine wait queues: the
    next block's center-replication chunks are emitted mid-block as PE
    filler, at CH=4-row granularity so their PSUM waits never head-block
    ready D-matmuls.
"""

import contextlib
import sys

import numpy as np

sys.path.insert(0, "/opt/trn_rl_repo")

import ml_dtypes

from concourse import bass, mybir, tile
from concourse.bass_utils import run_bass_kernel_spmd

# ---------------------------------------------------------------- constants
B, CIN, COUT, CG, H, W = 8, 64, 64, 16, 128, 128
KS, PAD = 3, 1
HP, WP = H + 2 * PAD, W + 2 * PAD  # 130, 130
NCORES = 8

R = 8                       # output rows per block
HGRP = 8                    # rows per psum group
CH = 4                      # output rows per matmul chunk (N = 4*128 = 512)

# non-center taps p=3i+j, p != 4, in reference order
TAPS = [(p // 3, p % 3) for p in range(9) if p != 4]
NT = len(TAPS)              # 8
CTR_I, CTR_J = 1, 1

# constants tile layout (free-dim columns)
CST_WSTK = 0                # [0:256)   wstk: col 64*g + o, part 16*t + c
CST_WCTR = 256              # [256:320) wctr: col o, part c  (64 parts)
CST_LHSD = 320              # [320:448) lhsd: block-diag -0.5
CST_BIAS = 448              # [448:449) bias  (64 parts)
CST_REP = 449               # [449:577) rep16: center-replication lhsT (16 parts)
CST_W = 577

F32 = mybir.dt.float32
BF = mybir.dt.bfloat16
NPBF = ml_dtypes.bfloat16

_cache = {}


# ---------------------------------------------------------------- bass build
def _build_nc():
    nc = bass.Bass(
        "TRN2",
        target_bir_lowering=False,
        debug=False,
        enable_asserts=False,
        num_devices=NCORES,
    )

    xs4_d = nc.dram_tensor("xs4", [128, 4, H, W], BF, kind="ExternalInput").ap()
    gs_d = nc.dram_tensor("gs", [128, H, W], BF, kind="ExternalInput").ap()
    gpc_d = nc.dram_tensor("gpc", [CG, H, W], BF, kind="ExternalInput").ap()
    xpb_d = nc.dram_tensor("xpb", [CIN, HP, WP], BF, kind="ExternalInput").ap()
    cst_d = nc.dram_tensor("cst", [128, CST_W], BF, kind="ExternalInput").ap()
    out_d = nc.dram_tensor("out", [COUT, H, W], BF, kind="ExternalOutput").ap()

    NBLK = H // R  # 8 blocks of 16 rows

    with tile.TileContext(nc) as tc:
        with contextlib.ExitStack() as ctx:
            cstp = ctx.enter_context(tc.tile_pool(name="cstp", bufs=1))
            bin_ = ctx.enter_context(tc.tile_pool(name="bin", bufs=5))
            blk = ctx.enter_context(tc.tile_pool(name="blk", bufs=3))
            cnk = ctx.enter_context(tc.tile_pool(name="cnk", bufs=3))
            psg = ctx.enter_context(tc.tile_pool(name="psg", bufs=2, space="PSUM"))
            psd = ctx.enter_context(tc.tile_pool(name="psd", bufs=4, space="PSUM"))
            pso = ctx.enter_context(tc.tile_pool(name="pso", bufs=1, space="PSUM"))

            cst = cstp.tile([128, CST_W], BF, name="cst")
            nc.sync.dma_start(cst[:], cst_d[:])
            wstk = [cst[:, CST_WSTK + 64 * g : CST_WSTK + 64 * (g + 1)]
                    for g in range(4)]
            wctr = cst[0:CIN, CST_WCTR : CST_WCTR + COUT]
            lhsd = cst[:, CST_LHSD : CST_LHSD + 128]
            rep16 = cst[0:CG, CST_REP : CST_REP + 128]
            bias32 = cstp.tile([COUT, 1], F32, name="bias32")
            nc.vector.tensor_copy(bias32[:], cst[0:COUT, CST_BIAS : CST_BIAS + 1])

            # whole-image replicated guide-center buffer. Two half-image gpc
            # DMAs (issued up front / at the halfway block); the PE-matmul ->
            # Pool-evict replication chunks are interleaved into each block's
            # phase A so they never head-block the in-order PE queue.
            gpcw = ctx.enter_context(tc.tile_pool(name="gpcw", bufs=1))
            gcs = gpcw.tile([128, H, W], BF, name="gcs")
            H2 = H // 2
            gpc2 = [gpcw.tile([CG, H2, W], BF, name=f"gpc2{i}") for i in range(2)]
            for i in range(2):
                nc.sync.dma_start(gpc2[i][:], gpc_d[:, H2 * i : H2 * (i + 1), :])

            def gcs_chunks(b):
                r0 = R * b
                gp = gpc2[r0 // H2]
                for v in range(R // CH):
                    gcr = psg.tile([128, CH, W], F32, name="gcr")
                    nc.tensor.matmul(
                        gcr[:],
                        rep16,
                        gp[:, (r0 % H2) + CH * v : (r0 % H2) + CH * (v + 1), :],
                        start=True,
                        stop=True,
                    )
                    nc.scalar.activation(
                        gcs[:, r0 + CH * v : r0 + CH * (v + 1), :],
                        gcr[:],
                        mybir.ActivationFunctionType.Copy,
                    )

            opsb = pso.tile([128, HGRP, W], F32, name="opsb")

            def block(b):
                r0 = R * b
                gs = bin_.tile([128, R, W], BF, name="gs")
                nc.gpsimd.dma_start(gs[:], gs_d[:, r0 : r0 + R, :])
                xs4 = bin_.tile([128, 4, R, W], BF, name="xs4")
                nc.sync.dma_start(xs4[:], xs4_d[:, :, r0 : r0 + R, :])
                xpb = bin_.tile([CIN, R + 2, WP], BF, name="xpb")
                nc.sync.dma_start(xpb[:], xpb_d[:, r0 : r0 + R + 2, :])

                nc.vector.tensor_sub(gs[:], gs[:], gcs[:, r0 : r0 + R, :])
                sq = blk.tile([128, R, W], BF, name="sq")
                nc.vector.tensor_mul(sq[:], gs[:], gs[:])
                e8 = blk.tile([128, R, W], BF, name="e8")
                osb = blk.tile([COUT, R, W], BF, name="osb")

                for h in range(R // HGRP):
                    hr = HGRP * h
                    for q in range(HGRP // CH):
                        dps = psd.tile([128, CH, W], F32, name="dps")
                        nc.tensor.matmul(
                            dps[:],
                            lhsd,
                            sq[:, hr + CH * q : hr + CH * (q + 1), :],
                            start=True,
                            stop=True,
                        )
                        nc.scalar.activation(
                            e8[:, hr + CH * q : hr + CH * (q + 1), :],
                            dps[:],
                            mybir.ActivationFunctionType.Exp,
                        )

                    ys = []
                    for g in range(4):
                        yt = cnk.tile([128, HGRP, W], BF, name=f"y{g}")
                        nc.vector.tensor_mul(
                            yt[:],
                            xs4[:, g, hr : hr + HGRP, :],
                            e8[:, hr : hr + HGRP, :],
                        )
                        ys.append(yt)

                    # consecutive blocks alternate partition halves of the
                    # single hoisted PSUM tile (WAR distance 2 blocks)
                    ops = opsb[64 * (b % 2) : 64 * (b % 2) + COUT, :, :]
                    # lhsT-major order: one Ldweights per stationary tensor
                    for g in range(4):
                        for q in range(HGRP // CH):
                            nc.tensor.matmul(
                                ops[:, CH * q : CH * (q + 1), :],
                                wstk[g],
                                ys[g][:, CH * q : CH * (q + 1), :],
                                start=(g == 0),
                                stop=False,
                            )
                    for q in range(HGRP // CH):
                        nc.tensor.matmul(
                            ops[:, CH * q : CH * (q + 1), :],
                            wctr,
                            xpb[:, 1 + hr + CH * q : 1 + hr + CH * (q + 1),
                                1 : 1 + W],
                            start=False,
                            stop=True,
                        )

                    nc.scalar.activation(
                        osb[:, hr : hr + HGRP, :],
                        ops[:],
                        mybir.ActivationFunctionType.Relu,
                        bias=bias32[:],
                    )
                    nc.scalar.dma_start(
                        out_d[:, r0 + hr : r0 + hr + HGRP, :],
                        osb[:, hr : hr + HGRP, :],
                    )

                    if h == 0 and b + 1 < NBLK:
                        # next block's guide-center replication as PE filler
                        gcs_chunks(b + 1)

            gcs_chunks(0)
            for b in range(NBLK):
                block(b)

    _split_waits(nc)
    return nc


_SKIP_SPLIT = {"InstCall", "InstUnconditionalBranch", "InstEventSemaphore"}


def _split_waits(nc):
    """Walrus's PSEUDO_DMA_DIRECT2D (and friends) carry a single sync-wait
    slot; Tile can attach several. Peel extra waits onto single-wait
    EventSemaphore instructions on the same engine immediately before the
    instruction (classic raw-bass wait-then-issue pattern)."""
    nopctr = [0]
    scratch_id = max(int(k) for k in nc.m.ant_sem_names) + 1
    nc.m.ant_sem_names[str(scratch_id)] = ["waitnop_scratch"]

    def mk_nop(engine, wait):
        nopctr[0] += 1
        nop = mybir.InstEventSemaphore(
            name=f"I-waitnop-{nopctr[0]}", ins=[], outs=[]
        )
        nop.engine = engine
        upd = mybir.SyncUpdate(
            sync_type="semaphore",
            id=scratch_id,
            ant_name="waitnop_scratch",
            update_mode="sem-add-imm",
            update_value=0,
            update_reg=None,
        )
        nop.sync_info = mybir.SyncInfo(on_wait=[wait], on_update=[upd])
        return nop

    for f in nc.m.functions:
        for blk_ in f.blocks:
            out = []
            for inst in blk_.instructions:
                si = inst.sync_info
                if (
                    si is not None
                    and si.on_wait
                    and len(si.on_wait) > 1
                    and type(inst).__name__ not in _SKIP_SPLIT
                ):
                    waits = list(si.on_wait)
                    for w in waits[:-1]:
                        out.append(mk_nop(inst.engine, w))
                    inst.sync_info = mybir.SyncInfo(
                        on_wait=[waits[-1]], on_update=list(si.on_update)
                    )
                out.append(inst)
            blk_.instructions[:] = out


def _get_nc():
    if "nc" not in _cache:
        _cache["nc"] = _build_nc()
    return _cache["nc"]


# ---------------------------------------------------------------- host side
def _prep_inputs(x, guide, weight, bias):
    x = np.asarray(x, dtype=np.float32)
    guide = np.asarray(guide, dtype=np.float32)
    weight = np.asarray(weight, dtype=np.float32)
    bias = np.asarray(bias, dtype=np.float32)

    xpb = np.pad(x, ((0, 0), (0, 0), (PAD, PAD), (PAD, PAD))).astype(NPBF)
    gp = np.pad(guide, ((0, 0), (0, 0), (PAD, PAD), (PAD, PAD))).astype(NPBF)

    # pre-stacked im2col tap tensors (pure layout, no arithmetic)
    xs4 = np.empty((B, 128, 4, H, W), dtype=NPBF)
    gs = np.empty((B, 128, H, W), dtype=NPBF)
    for t, (ti, tj) in enumerate(TAPS):
        for g in range(4):
            xs4[:, 16 * t : 16 * t + 16, g] = xpb[
                :, 16 * g : 16 * g + 16, ti : ti + H, tj : tj + W
            ]
        gs[:, 16 * t : 16 * t + 16] = gp[:, :, ti : ti + H, tj : tj + W]
    gpc = np.ascontiguousarray(gp[:, :, CTR_I : CTR_I + H, CTR_J : CTR_J + W])

    cst = np.zeros((128, CST_W), dtype=np.float32)
    for g in range(4):
        for t, (ti, tj) in enumerate(TAPS):
            cst[16 * t : 16 * t + 16,
                CST_WSTK + 64 * g : CST_WSTK + 64 * (g + 1)] = weight[
                :, 16 * g : 16 * g + 16, ti, tj
            ].T
    cst[0:CIN, CST_WCTR : CST_WCTR + COUT] = weight[:, :, CTR_I, CTR_J].T
    for t in range(NT):
        cst[16 * t : 16 * t + 16, CST_LHSD + 16 * t : CST_LHSD + 16 * t + 16] = (
            -0.5 * np.eye(16) @ np.ones((16, 16))
        )
    cst[0:COUT, CST_BIAS] = bias
    for c in range(CG):
        for t in range(NT):
            cst[c, CST_REP + 16 * t + c] = 1.0
    cst = cst.astype(NPBF)

    in_maps = []
    for i in range(NCORES):
        in_maps.append(
            {
                "xs4": np.ascontiguousarray(xs4[i]),
                "gs": np.ascontiguousarray(gs[i]),
                "gpc": gpc[i],
                "xpb": np.ascontiguousarray(xpb[i]),
                "cst": cst,
            }
        )
    return in_maps


def _run(in_maps, trace=False, **kw):
    nc = _get_nc()
    last = None
    for attempt in range(3):
        try:
            res = run_bass_kernel_spmd(
                nc, in_maps, list(range(NCORES)), trace=trace, **kw
            )
            break
        except Exception as e:  # wedged device: wait and retry
            last = e
            import time as _t

            _t.sleep(20 * (attempt + 1))
    else:
        raise last
    out = np.stack([res.results[i]["out"] for i in range(NCORES)], axis=0)
    return out.astype(np.float32), res


def kernel(x, guide, weight, bias):
    in_maps = _prep_inputs(x, guide, weight, bias)
    out, _ = _run(in_maps)
    return out
